# revision 3
# baseline (speedup 1.0000x reference)
"""Trainium2 Bass kernel for nn_ConvexLayer4 (per-feature 2->32->1 MLP).

Math (reference reduces to this; noise / h_higher / h_lower are dead code):
    u      = ln(10 * x)                        (na = ln(0.1/x) = -u)
    out[b,f] = sum_h sigmoid(-W1[f,0,h]*u[b,f] + k[f]*W1[f,1,h] + b1[f,h])
               * W2[f,h] + b2[f]               (x in [0,1) so the mask is 1)

Sharding: pure data parallel -- 512 batch rows per core. Input shards are
contiguous row slices of the full [4096, 256] array and the output shards
concatenate back to the full output, so the host does no packing at all.

Per-core dataflow (all fp32):
    DMA  x rows -> SBUF xb [128, 4*256]          (partition = b%128)
    PE   8 transposes -> PSUM, feature-major chunks [128f, 512b]
    ACT  u = Ln(10x + 1e-30)
    64 h-steps (2 feature chunks x 32 hidden):
        ACT s = Sigmoid(scale[f,h]*u + bias[f,h])   (per-partition consts)
        DVE acc = s*W2[f,h] + acc                   (fused scalar_tensor_tensor)
    PE   8 transposes back to batch-major, ACT copy PSUM->SBUF
    DMA  out rows in natural [512, 256] layout
"""
import sys
import time

sys.path.insert(0, "/opt/trn_rl_repo")

import numpy as np

import concourse.bacc as bacc
import concourse.bass as bass
import concourse.tile as tile
from concourse import mybir

B, F, H = 4096, 256, 32
NCORES = 8
BL = B // NCORES          # 512 batch rows per core
NBB = BL // 128           # 4 row blocks of 128
NFC = F // 128            # 2 feature chunks of 128
FP32 = mybir.dt.float32
NCOL = 200                # consts columns (195 used)

_nc_cache = {}


def build_nc(reps=1):
    """Build (and compile) the SPMD program. `reps` statically repeats the
    compute section for slope-based HW timing; reps=1 is the production run."""
    if reps in _nc_cache:
        return _nc_cache[reps]
    nc = bacc.Bacc("TRN2", target_bir_lowering=False, num_devices=NCORES)

    xin = nc.dram_tensor("xin", [BL, F], FP32, kind="ExternalInput")
    consts = nc.dram_tensor("consts", [128, NCOL], FP32, kind="ExternalInput")
    outp = nc.dram_tensor("outp", [BL, F], FP32, kind="ExternalOutput")

    with tile.TileContext(nc) as tc:
        with (
            tc.tile_pool(name="fixed", bufs=1) as fixed,
            tc.tile_pool(name="upool", bufs=2) as upool,
            tc.tile_pool(name="spool", bufs=3) as spool,
            tc.tile_pool(name="apool", bufs=2) as apool,
            tc.tile_pool(name="opool", bufs=2) as opool,
            tc.tile_pool(name="psx", bufs=2, space="PSUM") as psx_pool,
            tc.tile_pool(name="pso", bufs=2, space="PSUM") as pso_pool,
        ):
            c_sb = fixed.tile([128, NCOL], FP32)
            nc.sync.dma_start(out=c_sb, in_=consts[:, :])

            # x rows -> SBUF: partition = b%128, col = bb*256 + f
            xb = fixed.tile([128, NBB * F], FP32)
            x_ap = xin[:, :]
            xsrc = bass.AP(tensor=x_ap.tensor, offset=0,
                           ap=[[F, 128], [128 * F, NBB], [1, F]])
            nc.sync.dma_start(out=xb, in_=xsrc)

            # 128x128 identity (for PE transposes): ones masked to p == col
            ident = fixed.tile([128, 128], FP32)
            nc.gpsimd.memset(ident, 1.0)
            nc.gpsimd.affine_select(
                out=ident, in_=ident, pattern=[[-1, 128]],
                compare_op=mybir.AluOpType.is_equal, fill=0.0,
                base=0, channel_multiplier=1)

            for _rep in range(reps):
                us = []
                # input transposes + Ln for both chunks up front so the PE
                # is done before the ACT/DVE main loop starts
                for c in range(NFC):
                    ps = psx_pool.tile([128, BL], FP32, tag="psx")
                    for bb in range(NBB):
                        nc.tensor.transpose(
                            out=ps[:, 128 * bb:128 * (bb + 1)],
                            in_=xb[:, 256 * bb + 128 * c:256 * bb + 128 * c + 128],
                            identity=ident)
                    u = upool.tile([128, BL], FP32, tag="u")
                    nc.scalar.activation(
                        out=u, in_=ps, func=mybir.ActivationFunctionType.Ln,
                        bias=c_sb[:, 194:195], scale=10.0)
                    us.append(u)

                for c in range(NFC):
                    u = us[c]
                    acc = apool.tile([128, BL], FP32, tag="acc")
                    for h in range(H):
                        col = 32 * c + h
                        s = spool.tile([128, BL], FP32, tag="s")
                        nc.scalar.activation(
                            out=s, in_=u,
                            func=mybir.ActivationFunctionType.Sigmoid,
                            bias=c_sb[:, 64 + col:65 + col],
                            scale=c_sb[:, col:col + 1])
                        if h == 0:
                            # acc = s*W2[f,0] + b2[f]
                            nc.vector.tensor_scalar(
                                out=acc, in0=s,
                                scalar1=c_sb[:, 128 + col:129 + col],
                                scalar2=c_sb[:, 192 + c:193 + c],
                                op0=mybir.AluOpType.mult,
                                op1=mybir.AluOpType.add)
                        else:
                            # acc = s*W2[f,h] + acc
                            nc.vector.scalar_tensor_tensor(
                                out=acc, in0=s,
                                scalar=c_sb[:, 128 + col:129 + col],
                                in1=acc,
                                op0=mybir.AluOpType.mult,
                                op1=mybir.AluOpType.add)
                    # transpose acc back to batch-major and store
                    for bb in range(NBB):
                        po = pso_pool.tile([128, 128], FP32, tag="pso")
                        nc.tensor.transpose(
                            out=po, in_=acc[:, 128 * bb:128 * (bb + 1)],
                            identity=ident)
                        ost = opool.tile([128, 128], FP32, tag="ost")
                        nc.scalar.copy(out=ost, in_=po)
                        o_ap = outp[:, :]
                        odst = bass.AP(tensor=o_ap.tensor,
                                       offset=128 * bb * F + 128 * c,
                                       ap=[[F, 128], [1, 128]])
                        nc.sync.dma_start(out=odst, in_=ost)
    nc.compile()
    _nc_cache[reps] = nc
    return nc


class SpmdRunner:
    """Persistent jitted SPMD executor (built once per program)."""

    def __init__(self, nc, n_cores):
        import jax
        import numpy as _np
        from jax.sharding import Mesh, PartitionSpec
        from jax.experimental.shard_map import shard_map
        from concourse import bass2jax, mybir as _mybir

        bass2jax.install_neuronx_cc_hook()
        self.nc = nc
        self.n_cores = n_cores
        in_names, out_names, out_avals, zero_shapes = [], [], [], []
        partition_name = (nc.partition_id_tensor.name
                          if nc.partition_id_tensor else None)
        for alloc in nc.m.functions[0].allocations:
            if not isinstance(alloc, _mybir.MemoryLocationSet):
                continue
            name = alloc.memorylocations[0].name
            if alloc.kind == "ExternalInput":
                if name != partition_name:
                    in_names.append(name)
            elif alloc.kind == "ExternalOutput":
                out_names.append(name)
                out_avals.append(jax.core.ShapedArray(
                    tuple(alloc.tensor_shape), _mybir.dt.np(alloc.dtype)))
                zero_shapes.append((tuple(alloc.tensor_shape),
                                    _mybir.dt.np(alloc.dtype)))
        self.in_names, self.out_names = list(in_names), out_names
        self.out_avals, self.zero_shapes = out_avals, zero_shapes
        n_params, n_outs = len(in_names), len(out_names)
        all_names = list(in_names) + list(out_names)
        if partition_name is not None:
            all_names.append(partition_name)

        def _body(*args):
            operands = list(args)
            if partition_name is not None:
                operands.append(bass2jax.partition_id_tensor())
            return tuple(bass2jax._bass_exec_p.bind(
                *operands,
                out_avals=tuple(out_avals),
                in_names=tuple(all_names),
                out_names=tuple(out_names),
                lowering_input_output_aliases=(),
                sim_require_finite=True,
                sim_require_nnan=True,
                nc=nc,
            ))

        devices = jax.devices()[:n_cores]
        mesh = Mesh(_np.asarray(devices), ("core",))
        in_specs = (PartitionSpec("core"),) * (n_params + n_outs)
        out_specs = (PartitionSpec("core"),) * n_outs
        self._fn = jax.jit(
            shard_map(_body, mesh=mesh, in_specs=in_specs,
                      out_specs=out_specs, check_rep=False),
            keep_unused=True,
        )
        self._zeros = [np.zeros((n_cores * s[0], *s[1:]), d)
                       for s, d in zero_shapes]

    def build_args(self, x, consts_tiled):
        args = []
        for name in self.in_names:
            args.append(x if name == "xin" else consts_tiled)
        return args + self._zeros

    def run_prepped(self, args):
        outs = self._fn(*args)
        import jax
        jax.block_until_ready(outs)
        return outs


_runner_cache = {}


def get_runner(reps=1):
    if reps not in _runner_cache:
        _runner_cache[reps] = SpmdRunner(build_nc(reps), NCORES)
    return _runner_cache[reps]


def prep_consts(k, W1, b1, W2, b2):
    """Per-partition constants, tiled for all cores: [8*128, NCOL]."""
    k = np.asarray(k, np.float32)
    W1 = np.asarray(W1, np.float32)
    b1 = np.asarray(b1, np.float32)
    W2 = np.asarray(W2, np.float32)
    b2 = np.asarray(b2, np.float32)
    A = W1[:, 0, :]                          # [256, 32]
    C = k[:, None] * W1[:, 1, :] + b1        # [256, 32]
    c0 = np.zeros((128, NCOL), np.float32)
    c0[:, 0:64] = (-A).reshape(2, 128, 32).transpose(1, 0, 2).reshape(128, 64)
    c0[:, 64:128] = C.reshape(2, 128, 32).transpose(1, 0, 2).reshape(128, 64)
    c0[:, 128:192] = W2.reshape(2, 128, 32).transpose(1, 0, 2).reshape(128, 64)
    c0[:, 192:194] = b2.reshape(2, 128).T
    c0[:, 194] = 1e-30
    return np.tile(c0, (NCORES, 1))


_consts_cache = {}


def consts_cached(k, W1, b1, W2, b2):
    import hashlib
    hsh = hashlib.blake2b(
        np.asarray(k, np.float32).tobytes()
        + np.asarray(W1, np.float32).tobytes()
        + np.asarray(b1, np.float32).tobytes()
        + np.asarray(W2, np.float32).tobytes()
        + np.asarray(b2, np.float32).tobytes(),
        digest_size=16).digest()
    hit = _consts_cache.get(hsh)
    if hit is None:
        hit = prep_consts(k, W1, b1, W2, b2)
        _consts_cache.clear()
        _consts_cache[hsh] = hit
    return hit


def kernel(inputs, noise=None, k=None, W1=None, b1=None, W2=None, b2=None):
    runner = get_runner(reps=1)
    x = np.ascontiguousarray(np.asarray(inputs, dtype=np.float32))
    cc = consts_cached(k, W1, b1, W2, b2)
    outs = runner.run_prepped(runner.build_args(x, cc))
    return np.asarray(outs[0])


def measure_hw_time_ns(x, cc, r_lo=1, r_hi=33, trials=7):
    """Slope-based HW kernel time: (t(r_hi) - t(r_lo)) / (r_hi - r_lo)."""
    run_lo, run_hi = get_runner(r_lo), get_runner(r_hi)
    args_lo = run_lo.build_args(x, cc)
    args_hi = run_hi.build_args(x, cc)
    run_lo.run_prepped(args_lo)
    run_hi.run_prepped(args_hi)
    t_lo, t_hi = [], []
    for _ in range(trials):
        t0 = time.perf_counter()
        run_lo.run_prepped(args_lo)
        t_lo.append(time.perf_counter() - t0)
        t0 = time.perf_counter()
        run_hi.run_prepped(args_hi)
        t_hi.append(time.perf_counter() - t0)
    dt = (min(t_hi) - min(t_lo)) / (r_hi - r_lo)
    return dt * 1e9


if __name__ == "__main__":
    rng = np.random.default_rng(0)
    x = rng.random((B, F), dtype=np.float32)
    kk = rng.standard_normal(F).astype(np.float32) * 0.1
    W1 = rng.standard_normal((F, 2, H)).astype(np.float32) * 0.5
    b1 = np.zeros((F, H), np.float32)
    W2 = rng.standard_normal((F, H)).astype(np.float32) * 0.2
    b2 = np.zeros((F,), np.float32)
    out = kernel(x, None, kk, W1, b1, W2, b2)
    print("kernel ran, out shape", out.shape)
